# revision 1
# baseline (speedup 1.0000x reference)
"""Trainium2 Bass kernel for the digit-conv model.

Math: y = relu(relu(conv3x3(x) @ W1 + b1) @ W2 + b2) @ W3 + b3.
The valid 3x3 conv is linear, so it folds into W1 on device:
feat = x @ A with A[u, q] sparse from conv_w, hence
W1eff = A @ W1 and y = mlp(x @ W1eff ...). The kernel computes
W1eff = A^T.T @ W1 on the tensor engine once (A^T is banded, so
all-zero blocks are statically skipped), then streams the batch
through the 3-layer MLP entirely as lhsT.T @ rhs matmuls with channels
on partitions and batch on the free dimension (no transposes needed:
x is supplied pre-transposed per shard, and every weight is already in
[K, M] layout).

Sharding: pure data parallel — batch split across 8 cores, weights
replicated. Host-side work is limited to layout (x transpose + shard +
bf16 cast, zero-padding, band extraction) and scattering the 9 conv
weights into the A^T matrix (no arithmetic).

All matmul operands are bf16: fp32 x (25.7 MB/core) would be DMA-bound
(~250 GB/s/core effective -> ~103 us, above the ~97 us tensor-engine
floor). PSUM accumulation is fp32; biases are applied in fp32 from
PSUM. (f32r was measured more accurate but cannot be mixed with bf16:
walrus inserts round-to-fp32r passes over DMA-loaded f32r tiles that
corrupt neighboring tiles in a mixed-dtype program.)

DMA instruction count is minimized (one multi-tile DMA per logical
group via 3D access patterns): each dma_start costs ~650 ns of serial
issue on its queue engine, which dominated startup in earlier
revisions. The packed fold tensor and x supers go on the Sync (HWDGE)
path; small constants (w2/w3/bias) go through GpSimd (SWDGE) so they
don't serialize with them. Full-width (N=512) dummy matmuls on a
memset tile warm the PE clock-gate before real work arrives — HAM
watches array-busy duty cycle, so narrow warmup matmuls do NOT trip
it. The batch super-block widths ramp 256 -> 1536 so the main loop
starts right behind the fold DMA, and the final 256-wide super halves
the exposed end-of-stream dependency chain. A small post-fold filler
block of dummy matmuls bridges the fold->x_0 DMA wait so an unlucky
per-core HAM window phase cannot re-throttle the clock (this cut the
worst-core exec from ~119 to ~116 us; per-core spread ~113.5-116.3).

Measured on the 8 axon TRN2 cores: ~113-118 us HW exec at 2.4 GHz
(the chip drifts to a 2.0 GHz power state at times: +20%), ~5.2e-3
scale-relative error vs the fp32 reference. Breakdown: ~7 us fixed
NEFF preamble, ~5 us DMA-bound ramp-in (fold + first x super),
~93 us matmul stream with <1.5 us of gaps (400 main matmuls x 216 ns
is the hard floor: ceil(784/128)*ceil(300/128)+ceil(300/128)+1 = 25
bank passes per 512 batch columns), ~11 us fixed walrus
semaphore-teardown and drain barrier.
"""

import ml_dtypes
import numpy as np

import concourse.tile as tile
from concourse import bacc, mybir
from concourse import bass_utils

N_CORES = 8
B = 65536
BC = B // N_CORES  # 8192 rows per core
U = 784            # input features (28*28)
Q = 676            # conv outputs (26*26)
QP = 768           # q padded to 6 full tiles of 128
H1, H2, H3 = 300, 100, 10
NB = 512           # batch columns per PSUM block (one bank of fp32)
SUP = 1536         # max batch columns per DMA super-block
SUP_WIDTHS = [256, 512, 1024, 1536, 1536, 1536, 1536, 256]
assert sum(SUP_WIDTHS) == BC
KT = 112           # u-dim k-tile (784 = 7*112)
NKT = 7
MC = 100           # layer-1 output chunk (300 = 3*100)
NMC = 3
ABW = 336          # amat band width (3 u-chunks), fixed for all q-tiles

_prog_cache = {}


def _fold_bands():
    """Static block-sparsity of A^T [Q, U]: per 128-row q-tile, the nonzero
    columns lie in a band; returns per-tile (q0, p_real, c_lo, c_hi) with the
    band given in whole 112-wide u-chunks (at most 3 chunks wide)."""
    bands = []
    for qt in range(QP // 128):
        q0 = qt * 128
        p_real = min(128, Q - q0)
        i_lo = q0 // 26
        i_hi = (q0 + p_real - 1) // 26
        u_lo = 28 * i_lo
        u_hi = min(U, 28 * (i_hi + 2) + 28)   # exclusive upper bound
        c_lo = u_lo // KT
        c_hi = (u_hi + KT - 1) // KT          # exclusive chunk bound
        assert c_hi - c_lo <= ABW // KT
        bands.append((q0, p_real, c_lo, c_hi))
    return bands


def _build_program():
    f32 = mybir.dt.float32
    bf16 = mybir.dt.bfloat16
    relu = mybir.ActivationFunctionType.Relu
    alu_add = mybir.AluOpType.add
    alu_max = mybir.AluOpType.max

    nc = bacc.Bacc(
        "TRN2", target_bir_lowering=False, debug=False, num_devices=N_CORES
    )

    nqt = QP // 128
    xT_d = nc.dram_tensor("xT", [U, BC], bf16, kind="ExternalInput").ap()
    FW = ABW + H1  # 636: packed [amat band | w1] row width
    fold_d = nc.dram_tensor("fold", [QP, FW], bf16, kind="ExternalInput").ap()
    w2_d = nc.dram_tensor("w2", [H1, H2], bf16, kind="ExternalInput").ap()
    w3_d = nc.dram_tensor("w3", [H2, H3], bf16, kind="ExternalInput").ap()
    bias_d = nc.dram_tensor("bias", [MC, 5], f32, kind="ExternalInput").ap()
    yT_d = nc.dram_tensor("yT", [H3, BC], f32, kind="ExternalOutput").ap()

    bands = _fold_bands()

    with tile.TileContext(nc) as tc:
        with tc.tile_pool(name="const", bufs=1) as cpool, \
             tc.tile_pool(name="xp", bufs=5) as xpool, \
             tc.tile_pool(name="hp", bufs=2) as hpool, \
             tc.tile_pool(name="yp", bufs=2) as ypool, \
             tc.tile_pool(name="ps1", bufs=6, space="PSUM") as ps1p, \
             tc.tile_pool(name="ps2", bufs=2, space="PSUM") as ps2p:

            # ---- HAM warmup: dummy matmuls on a memset tile so the PE
            # clock-gate releases before the real work arrives ----
            warm_sb = cpool.tile([128, 512], bf16)
            nc.vector.memset(warm_sb[:], 0.0)
            for wi in range(15):
                pw = ps2p.tile([128, NB], f32, tag="l2", name=f"pwarm_{wi}")
                nc.tensor.matmul(pw[:], warm_sb[:, :128], warm_sb[:],
                                 start=True, stop=True)

            # ---- constants into SBUF (one merged DMA per group, on the
            # SWDGE path so they don't block x-load issue on HWDGE) ----
            fold_sb = cpool.tile([128, nqt * FW], bf16)
            nc.sync.dma_start(
                fold_sb[:].rearrange("p (q c) -> p q c", c=FW),
                fold_d.rearrange("(q p) c -> p q c", p=128),
            )
            w2_sb = cpool.tile([MC, NMC * H2], bf16)
            nc.gpsimd.dma_start(
                w2_sb[:].rearrange("p (k c) -> p k c", c=H2),
                w2_d.rearrange("(k p) c -> p k c", p=MC),
            )
            w3_sb = cpool.tile([H2, H3], bf16)
            nc.gpsimd.dma_start(w3_sb[:], w3_d)
            bias_sb = cpool.tile([MC, 5], f32)
            nc.gpsimd.dma_start(bias_sb[:], bias_d)

            # ---- fold the conv into W1: W1eff[u, c] = (A^T).T @ W1 ----
            # Only q-tiles whose band covers the u-chunk contribute; the
            # rest are all-zero blocks of the banded A^T and are skipped.
            # (fold PSUM shares the l1 slot group: same tag, bank-sized)
            w1eff_sb = cpool.tile([KT, NKT * H1], bf16)
            for ut in range(NKT):
                parts = [qt for qt, (_, _, c_lo, c_hi) in enumerate(bands)
                         if c_lo <= ut < c_hi]
                assert parts
                pf = ps1p.tile([KT, NB], f32, tag="l1", name=f"pfold_{ut}")
                for idx, qt in enumerate(parts):
                    _, _, c_lo, _ = bands[qt]
                    off = qt * FW + (ut - c_lo) * KT
                    nc.tensor.matmul(
                        pf[:, :H1],
                        fold_sb[:, off:off + KT],
                        fold_sb[:, qt * FW + ABW:(qt + 1) * FW],
                        start=(idx == 0),
                        stop=(idx == len(parts) - 1),
                    )
                nc.vector.tensor_copy(
                    w1eff_sb[:, ut * H1:(ut + 1) * H1], pf[:, :H1])

            # ---- post-fold filler: bridge the fold->x_0 DMA wait so an
            # unlucky HAM MID-window phase can't re-throttle the PE on
            # cores whose x ramp lands late ----
            for wi in range(4):
                pw = ps2p.tile([128, NB], f32, tag="l2", name=f"pfill_{wi}")
                nc.tensor.matmul(pw[:], warm_sb[:, :128], warm_sb[:],
                                 start=True, stop=True)

            # ---- main pipeline over batch super-blocks ----
            # L2/L3 of each block are emitted AFTER the next block's
            # L1+relu, so the PE reaches them with their ACT dependencies
            # long satisfied (removes ~80ns stalls at the chunk->L2 edge).
            def emit_l2l3(h1_sb, nb, y_sb, pb, y_start, y_sw, last_of_sup):
                p2 = ps2p.tile([H2, nb], f32, tag="l2",
                               name=f"p2_{h1_sb.tensor.name}",
                               padded_shape=[H2, NB])
                for k2 in range(3):
                    nc.tensor.matmul(
                        p2[:], w2_sb[:, k2 * H2:(k2 + 1) * H2],
                        h1_sb[:, k2 * nb:(k2 + 1) * nb],
                        start=(k2 == 0), stop=(k2 == 2),
                    )
                h2 = hpool.tile([H2, nb], bf16, tag="h2",
                                name=f"h2_{h1_sb.tensor.name}",
                                padded_shape=[H2, NB])
                nc.vector.tensor_scalar(
                    h2[:], p2[:], bias_sb[:, 3:4], 0.0, alu_add, alu_max
                )
                p3 = ps2p.tile([H3, nb], f32, tag="l2",
                               name=f"p3_{h1_sb.tensor.name}",
                               padded_shape=[H3, NB])
                nc.tensor.matmul(p3[:], w3_sb[:], h2[:],
                                 start=True, stop=True)
                nc.vector.tensor_scalar_add(
                    y_sb[:, pb * NB:pb * NB + nb], p3[:],
                    bias_sb[:H3, 4:5])
                if last_of_sup:
                    nc.sync.dma_start(
                        yT_d[:, y_start:y_start + y_sw], y_sb[:])

            pending = None
            sup_start = 0
            for sup, sw in enumerate(SUP_WIDTHS):
                xtile = xpool.tile([KT, NKT * sw], bf16, tag="x",
                                   name=f"xt_{sup}",
                                   padded_shape=[KT, NKT * SUP])
                nc.sync.dma_start(
                    xtile[:].rearrange("p (k c) -> p k c", c=sw),
                    xT_d[:, sup_start:sup_start + sw]
                    .rearrange("(k p) c -> p k c", p=KT),
                )

                y_sb = ypool.tile([H3, sw], f32, tag="y", name=f"y_{sup}",
                                  padded_shape=[H3, SUP])
                for pb in range((sw + NB - 1) // NB):
                    nb = min(NB, sw - pb * NB)
                    h1_sb = hpool.tile([MC, NMC * nb], bf16, tag="h1",
                                       name=f"h1_{sup}_{pb}",
                                       padded_shape=[MC, NMC * NB])
                    for mc in range(NMC):
                        p1 = ps1p.tile([MC, nb], f32, tag="l1",
                                       name=f"p1_{sup}_{pb}_{mc}",
                                       padded_shape=[MC, NB])
                        for kt in range(NKT):
                            nc.tensor.matmul(
                                p1[:],
                                w1eff_sb[:, kt * H1 + mc * MC:
                                         kt * H1 + (mc + 1) * MC],
                                xtile[:, kt * sw + pb * NB:
                                      kt * sw + pb * NB + nb],
                                start=(kt == 0),
                                stop=(kt == NKT - 1),
                            )
                        nc.scalar.activation(
                            h1_sb[:, mc * nb:(mc + 1) * nb], p1[:], relu,
                            bias=bias_sb[:, mc:mc + 1], scale=1.0,
                        )

                    if pending is not None:
                        emit_l2l3(*pending)
                    nblocks = (sw + NB - 1) // NB
                    pending = (h1_sb, nb, y_sb, pb, sup_start, sw,
                               pb == nblocks - 1)

                sup_start += sw

            emit_l2l3(*pending)

    nc.compile()
    return nc


def _build_amat_banded(conv_w: np.ndarray) -> np.ndarray:
    """Scatter the 9 conv weights into the banded A^T [QP, ABW]:
    A^T[q, u] = conv_w[ki, kj] for q = 26*i + j, u = 28*(i+ki) + (j+kj),
    stored per 128-row q-tile with columns [c_lo*KT, c_hi*KT) of the band."""
    amat = np.zeros((Q, U), np.float32)
    i = np.arange(26)
    j = np.arange(26)
    q = (26 * i[:, None] + j[None, :]).ravel()
    for ki in range(3):
        for kj in range(3):
            u = (28 * (i[:, None] + ki) + j[None, :] + kj).ravel()
            amat[q, u] = conv_w[ki, kj]
    banded = np.zeros((QP, ABW), np.float32)
    for (q0, p_real, c_lo, c_hi) in _fold_bands():
        w = (c_hi - c_lo) * KT
        banded[q0:q0 + p_real, :w] = amat[q0:q0 + p_real, c_lo * KT:c_hi * KT]
    return banded


def _make_in_maps(x, conv_w, W1, b1, W2, b2, W3, b3):
    bf = ml_dtypes.bfloat16
    xT = np.ascontiguousarray(x.T.astype(bf))  # [U, B] bf16
    foldpk = np.zeros((QP, ABW + H1), np.float32)
    foldpk[:, :ABW] = _build_amat_banded(conv_w)
    foldpk[:Q, ABW:] = np.asarray(W1, np.float32)
    foldpk = np.ascontiguousarray(foldpk.astype(bf))
    w2 = np.ascontiguousarray(np.asarray(W2, np.float32).astype(bf))
    w3 = np.ascontiguousarray(np.asarray(W3, np.float32).astype(bf))
    bias = np.zeros((MC, 5), np.float32)
    bias[:, :NMC] = np.asarray(b1, np.float32).reshape(NMC, MC).T
    bias[:, 3] = np.asarray(b2, np.float32)
    bias[:H3, 4] = np.asarray(b3, np.float32)
    in_maps = []
    for c in range(N_CORES):
        in_maps.append({
            "xT": np.ascontiguousarray(xT[:, c * BC:(c + 1) * BC]),
            "fold": foldpk,
            "w2": w2, "w3": w3,
            "bias": bias,
        })
    return in_maps


def kernel(x, conv_w, W1, b1, W2, b2, W3, b3):
    x = np.asarray(x, dtype=np.float32)
    conv_w = np.asarray(conv_w, dtype=np.float32)

    if "nc" not in _prog_cache:
        _prog_cache["nc"] = _build_program()
    nc = _prog_cache["nc"]

    in_maps = _make_in_maps(x, conv_w, W1, b1, W2, b2, W3, b3)
    res = bass_utils.run_bass_kernel_spmd(
        nc, in_maps, core_ids=list(range(N_CORES))
    )

    out = np.empty((B, H3), np.float32)
    for c in range(N_CORES):
        out[c * BC:(c + 1) * BC, :] = res.results[c]["yT"].T
    return out



# revision 2
# speedup vs baseline: 1.0984x; 1.0984x over previous
"""Trainium2 Bass kernel for the digit-conv model.

Math: y = relu(relu(conv3x3(x) @ W1 + b1) @ W2 + b2) @ W3 + b3.
The valid 3x3 conv is linear, so it folds into W1 on device:
feat = x @ A with A[u, q] sparse from conv_w, hence
W1eff = A @ W1 and y = mlp(x @ W1eff ...). The kernel computes
W1eff = A^T.T @ W1 on the tensor engine once (A^T is banded, so
all-zero blocks are statically skipped), then streams the batch
through the 3-layer MLP entirely as lhsT.T @ rhs matmuls with channels
on partitions and batch on the free dimension.

Sharding: pure data parallel - batch split across 8 cores, weights
replicated. Host-side work is limited to layout (x transpose + shard +
bf16 cast, zero-padding, band extraction) and scattering the 9 conv
weights into the A^T matrix (no arithmetic).

PE array packing (measured on HW via tile_position microbenchmarks:
col-tiled pairs overlap with delta ~3ns, mode switch ~105ns):
  - L1 output chunks are [128, 128, 44] instead of 3x100. The two
    128-chunks use the full array width. The 44-chunk of TWO adjacent
    batch blocks runs as a column-tiled (128,64) pair - block a at
    PSUM partitions 0:44 (array cols 0:63), block b at 64:108 (cols
    64:127) - so a pair of passes costs one pass. h1's third chunk for
    "b" blocks stays at partitions 64:108; bias/W2 rows are replicated
    there host-side so every downstream op stays lane-locked.
  - L2's third k-pass (contraction 44) for blocks a/b runs as a
    row-tiled (64,128) pair at row strips 0/64 - again one slot.
  - W3 is zero-padded [100, 112] so L3 runs in full (128,128) mode
    (the old [100,10] shape forced a (128,32) tile mode and two mode
    switches per block).
Per 1024-column pair this saves 8 of 50 matmul passes for ~4 mode
switches (~420ns); L1 array utilization rises from 68% to ~82%.

All matmul operands are bf16: fp32 x (25.7 MB/core) would be DMA-bound.
PSUM accumulation is fp32; biases are applied in fp32 from PSUM.

DMA instruction count is minimized (one multi-tile DMA per logical
group via 3D access patterns). The packed fold tensor and x supers go
on the Sync (HWDGE) path; small constants (w2/w3/bias) go through
GpSimd (SWDGE) so they don't serialize with them. Full-width (N=512)
dummy matmuls on a memset tile warm the PE clock-gate before real work
arrives. The batch super-block widths ramp 256 -> 512 -> 1024 so the
main loop starts right behind the fold DMA; steady supers are 1024 (=
one block pair); the final 256-wide super halves the exposed
end-of-stream dependency chain. A small post-fold filler block of
dummy matmuls bridges the fold->x_0 DMA wait so an unlucky per-core
HAM window phase cannot re-throttle the clock.
"""

import ml_dtypes
import numpy as np

import concourse.tile as tile
from concourse import bacc, mybir
from concourse import bass_utils

N_CORES = 8
B = 65536
BC = B // N_CORES  # 8192 rows per core
U = 784            # input features (28*28)
Q = 676            # conv outputs (26*26)
QP = 768           # q padded to 6 full tiles of 128
H1, H2, H3 = 300, 100, 10
W3P = 112          # w3 padded output width (keeps L3 in full PE mode)
NB = 512           # batch columns per PSUM block (one bank of fp32)
SUP = 1024         # max batch columns per DMA super-block
SUP_WIDTHS = [256, 512, 1024, 1024, 1024, 1024, 1024, 1024, 1024, 256]
assert sum(SUP_WIDTHS) == BC
KT = 112           # u-dim k-tile (784 = 7*112)
NKT = 7
# L1 output chunks: (start, size); the 44-chunk is the col-tiled one
MCS = [(0, 128), (128, 128), (256, 44)]
ABW = 336          # amat band width (3 u-chunks), fixed for all q-tiles

_prog_cache = {}


def _fold_bands():
    """Static block-sparsity of A^T [Q, U]: per 128-row q-tile, the nonzero
    columns lie in a band; returns per-tile (q0, p_real, c_lo, c_hi) with the
    band given in whole 112-wide u-chunks (at most 3 chunks wide)."""
    bands = []
    for qt in range(QP // 128):
        q0 = qt * 128
        p_real = min(128, Q - q0)
        i_lo = q0 // 26
        i_hi = (q0 + p_real - 1) // 26
        u_lo = 28 * i_lo
        u_hi = min(U, 28 * (i_hi + 2) + 28)   # exclusive upper bound
        c_lo = u_lo // KT
        c_hi = (u_hi + KT - 1) // KT          # exclusive chunk bound
        assert c_hi - c_lo <= ABW // KT
        bands.append((q0, p_real, c_lo, c_hi))
    return bands


def _build_program():
    f32 = mybir.dt.float32
    bf16 = mybir.dt.bfloat16
    relu = mybir.ActivationFunctionType.Relu
    alu_add = mybir.AluOpType.add
    alu_max = mybir.AluOpType.max

    nc = bacc.Bacc(
        "TRN2", target_bir_lowering=False, debug=False, num_devices=N_CORES
    )

    nqt = QP // 128
    xT_d = nc.dram_tensor("xT", [U, BC], bf16, kind="ExternalInput").ap()
    FW = ABW + H1  # 636: packed [amat band | w1] row width
    fold_d = nc.dram_tensor("fold", [QP, FW], bf16, kind="ExternalInput").ap()
    w2_d = nc.dram_tensor("w2", [128, 3 * H2], bf16, kind="ExternalInput").ap()
    w3_d = nc.dram_tensor("w3", [H2, W3P], bf16, kind="ExternalInput").ap()
    bias_d = nc.dram_tensor("bias", [128, 5], f32, kind="ExternalInput").ap()
    yT_d = nc.dram_tensor("yT", [H3, BC], f32, kind="ExternalOutput").ap()

    bands = _fold_bands()

    with tile.TileContext(nc) as tc:
        with tc.tile_pool(name="const", bufs=1) as cpool, \
             tc.tile_pool(name="xp", bufs=5) as xpool, \
             tc.tile_pool(name="h1p", bufs=4) as h1pool, \
             tc.tile_pool(name="h2p", bufs=2) as h2pool, \
             tc.tile_pool(name="yp", bufs=2) as ypool, \
             tc.tile_pool(name="ps1", bufs=5, space="PSUM") as ps1p, \
             tc.tile_pool(name="ps2", bufs=2, space="PSUM") as ps2p:

            # ---- HAM warmup: dummy matmuls on a memset tile so the PE
            # clock-gate releases before the real work arrives ----
            warm_sb = cpool.tile([128, 512], bf16)
            nc.vector.memset(warm_sb[:], 0.0)
            for wi in range(15):
                pw = ps2p.tile([128, NB], f32, tag="l2", name=f"pwarm_{wi}")
                nc.tensor.matmul(pw[:], warm_sb[:, :128], warm_sb[:],
                                 start=True, stop=True)

            # ---- constants into SBUF (one merged DMA per group, on the
            # SWDGE path so they don't block x-load issue on HWDGE) ----
            fold_sb = cpool.tile([128, nqt * FW], bf16)
            nc.sync.dma_start(
                fold_sb[:].rearrange("p (q c) -> p q c", c=FW),
                fold_d.rearrange("(q p) c -> p q c", p=128),
            )
            w2_sb = cpool.tile([128, 3 * H2], bf16)
            nc.gpsimd.dma_start(w2_sb[:], w2_d)
            w3_sb = cpool.tile([H2, W3P], bf16)
            nc.gpsimd.dma_start(w3_sb[:], w3_d)
            bias_sb = cpool.tile([128, 5], f32)
            nc.gpsimd.dma_start(bias_sb[:], bias_d)

            # ---- fold the conv into W1: W1eff[u, c] = (A^T).T @ W1 ----
            # Only q-tiles whose band covers the u-chunk contribute; the
            # rest are all-zero blocks of the banded A^T and are skipped.
            w1eff_sb = cpool.tile([KT, NKT * H1], bf16)
            for ut in range(NKT):
                parts = [qt for qt, (_, _, c_lo, c_hi) in enumerate(bands)
                         if c_lo <= ut < c_hi]
                assert parts
                pf = ps1p.tile([KT, NB], f32, tag="l1", name=f"pfold_{ut}")
                for idx, qt in enumerate(parts):
                    _, _, c_lo, _ = bands[qt]
                    off = qt * FW + (ut - c_lo) * KT
                    nc.tensor.matmul(
                        pf[:, :H1],
                        fold_sb[:, off:off + KT],
                        fold_sb[:, qt * FW + ABW:(qt + 1) * FW],
                        start=(idx == 0),
                        stop=(idx == len(parts) - 1),
                    )
                nc.vector.tensor_copy(
                    w1eff_sb[:, ut * H1:(ut + 1) * H1], pf[:, :H1])

            # ---- post-fold filler: bridge the fold->x_0 DMA wait so an
            # unlucky HAM MID-window phase can't re-throttle the PE on
            # cores whose x ramp lands late ----
            for wi in range(4):
                pw = ps2p.tile([128, NB], f32, tag="l2", name=f"pfill_{wi}")
                nc.tensor.matmul(pw[:], warm_sb[:, :128], warm_sb[:],
                                 start=True, stop=True)

            # ---- main pipeline over batch super-blocks ----
            # Blocks are processed in PAIRS (a, b): the 44-wide third L1
            # chunk of both blocks shares one PSUM bank as a column-tiled
            # (128,64) pair; L2's 44-contraction pass runs as a row-tiled
            # (64,128) pair. L2/L3 of each group are emitted AFTER the
            # next group's L1, so the PE reaches them with their ACT
            # dependencies long satisfied.
            def w1slice(kt, mc):
                c0, csz = MCS[mc]
                return w1eff_sb[:, kt * H1 + c0:kt * H1 + c0 + csz]

            def emit_chunk01(blk, mc):
                """Full-mode L1 pass for chunk 0/1 of one block + relu."""
                xtile, sw, pb, nb, h1 = (blk["xt"], blk["sw"], blk["pb"],
                                         blk["nb"], blk["h1"])
                p1 = ps1p.tile([128, nb], f32, tag="l1",
                               name=f"p1_{blk['id']}_{mc}",
                               padded_shape=[128, NB])
                for kt in range(NKT):
                    nc.tensor.matmul(
                        p1[:],
                        w1slice(kt, mc),
                        xtile[:, kt * sw + pb * NB:kt * sw + pb * NB + nb],
                        start=(kt == 0),
                        stop=(kt == NKT - 1),
                    )
                nc.scalar.activation(
                    h1[:, mc * nb:(mc + 1) * nb], p1[:], relu,
                    bias=bias_sb[:, mc:mc + 1], scale=1.0,
                )

            def emit_m2(group):
                """Col-tiled (128,64) passes for the 44-chunk: block a at
                PSUM 0:44 / array cols 0:63, block b at 64:108 / 64:127.
                A solo block only uses side a."""
                p1c = ps1p.tile([128, NB], f32, tag="l1",
                                name=f"p1c_{group[0]['id']}")
                sides = [(0, 44), (64, 108)]
                for kt in range(NKT):
                    for blk, (s0, s1) in zip(group, sides):
                        xtile, sw, pb, nb = (blk["xt"], blk["sw"],
                                             blk["pb"], blk["nb"])
                        nc.tensor.matmul(
                            p1c[s0:s1, :nb],
                            w1slice(kt, 2),
                            xtile[:, kt * sw + pb * NB:
                                  kt * sw + pb * NB + nb],
                            start=(kt == 0),
                            stop=(kt == NKT - 1),
                        )
                for blk, (s0, s1) in zip(group, sides):
                    nb = blk["nb"]
                    blk["h1s"] = s0
                    nc.scalar.activation(
                        blk["h1"][s0:s1, 2 * nb:3 * nb], p1c[s0:s1, :nb],
                        relu, bias=bias_sb[s0:s1, 2:3], scale=1.0,
                    )

            def emit_l2l3(group):
                """L2 + L3 + output for a group of 1-2 blocks. The two full
                L2 k-passes per block are full-mode; the 44-contraction
                pass runs as a row-tiled (64,128) pair at strips 0/64."""
                p2 = []
                for blk in group:
                    p2.append(ps2p.tile([H2, blk["nb"]], f32, tag="l2",
                                        name=f"p2_{blk['id']}",
                                        padded_shape=[H2, NB]))
                for k2 in range(2):
                    for blk, p in zip(group, p2):
                        nb = blk["nb"]
                        nc.tensor.matmul(
                            p[:], w2_sb[:, k2 * H2:(k2 + 1) * H2],
                            blk["h1"][:, k2 * nb:(k2 + 1) * nb],
                            start=(k2 == 0), stop=False,
                        )
                for blk, p in zip(group, p2):
                    nb = blk["nb"]
                    s0 = blk["h1s"]
                    nc.tensor.matmul(
                        p[:], w2_sb[s0:s0 + 44, 2 * H2:3 * H2],
                        blk["h1"][s0:s0 + 44, 2 * nb:3 * nb],
                        start=False, stop=True,
                    )
                h2s = []
                for blk, p in zip(group, p2):
                    h2 = h2pool.tile([H2, blk["nb"]], bf16, tag="h2",
                                     name=f"h2_{blk['id']}",
                                     padded_shape=[H2, NB])
                    nc.vector.tensor_scalar(
                        h2[:], p[:], bias_sb[:H2, 3:4], 0.0, alu_add, alu_max
                    )
                    h2s.append(h2)
                for blk, h2 in zip(group, h2s):
                    nb = blk["nb"]
                    p3 = ps2p.tile([W3P, nb], f32, tag="l2",
                                   name=f"p3_{blk['id']}",
                                   padded_shape=[W3P, NB])
                    nc.tensor.matmul(p3[:], w3_sb[:], h2[:],
                                     start=True, stop=True)
                    nc.vector.tensor_scalar_add(
                        blk["y"][:, blk["pb"] * NB:blk["pb"] * NB + nb],
                        p3[:H3, :], bias_sb[:H3, 4:5])
                    if blk["last"]:
                        nc.sync.dma_start(
                            yT_d[:, blk["y0"]:blk["y0"] + blk["sw"]],
                            blk["y"][:])

            pending = None
            open_blk = None
            sup_start = 0
            last_sup = len(SUP_WIDTHS) - 1
            for sup, sw in enumerate(SUP_WIDTHS):
                xtile = xpool.tile([KT, NKT * sw], bf16, tag="x",
                                   name=f"xt_{sup}",
                                   padded_shape=[KT, NKT * SUP])
                nc.sync.dma_start(
                    xtile[:].rearrange("p (k c) -> p k c", c=sw),
                    xT_d[:, sup_start:sup_start + sw]
                    .rearrange("(k p) c -> p k c", p=KT),
                )
                y_sb = ypool.tile([H3, sw], f32, tag="y", name=f"y_{sup}",
                                  padded_shape=[H3, SUP])
                nblocks = (sw + NB - 1) // NB
                for pb in range(nblocks):
                    nb = min(NB, sw - pb * NB)
                    h1 = h1pool.tile([128, 3 * nb], bf16, tag="h1",
                                     name=f"h1_{sup}_{pb}",
                                     padded_shape=[128, 3 * NB])
                    blk = {"xt": xtile, "sw": sw, "pb": pb, "nb": nb,
                           "h1": h1, "y": y_sb, "y0": sup_start,
                           "last": pb == nblocks - 1, "id": f"{sup}_{pb}"}
                    emit_chunk01(blk, 0)
                    emit_chunk01(blk, 1)
                    if open_blk is None and sup == last_sup:
                        emit_m2([blk])
                        if pending is not None:
                            emit_l2l3(pending)
                        pending = [blk]
                    elif open_blk is None:
                        open_blk = blk
                    else:
                        emit_m2([open_blk, blk])
                        if pending is not None:
                            emit_l2l3(pending)
                        pending = [open_blk, blk]
                        open_blk = None
                sup_start += sw
            assert open_blk is None
            emit_l2l3(pending)

    nc.compile()
    return nc


def _build_amat_banded(conv_w: np.ndarray) -> np.ndarray:
    """Scatter the 9 conv weights into the banded A^T [QP, ABW]:
    A^T[q, u] = conv_w[ki, kj] for q = 26*i + j, u = 28*(i+ki) + (j+kj),
    stored per 128-row q-tile with columns [c_lo*KT, c_hi*KT) of the band."""
    amat = np.zeros((Q, U), np.float32)
    i = np.arange(26)
    j = np.arange(26)
    q = (26 * i[:, None] + j[None, :]).ravel()
    for ki in range(3):
        for kj in range(3):
            u = (28 * (i[:, None] + ki) + j[None, :] + kj).ravel()
            amat[q, u] = conv_w[ki, kj]
    banded = np.zeros((QP, ABW), np.float32)
    for (q0, p_real, c_lo, c_hi) in _fold_bands():
        w = (c_hi - c_lo) * KT
        banded[q0:q0 + p_real, :w] = amat[q0:q0 + p_real, c_lo * KT:c_hi * KT]
    return banded


def _make_in_maps(x, conv_w, W1, b1, W2, b2, W3, b3):
    bf = ml_dtypes.bfloat16
    xT = np.ascontiguousarray(x.T.astype(bf))  # [U, B] bf16
    foldpk = np.zeros((QP, ABW + H1), np.float32)
    foldpk[:, :ABW] = _build_amat_banded(conv_w)
    foldpk[:Q, ABW:] = np.asarray(W1, np.float32)
    foldpk = np.ascontiguousarray(foldpk.astype(bf))
    # w2 packed as 3 k-chunk bands [128, 100] each; the 44-row third chunk
    # is replicated at partitions 64:108 for the "b" blocks of each pair
    W2f = np.asarray(W2, np.float32)
    w2pk = np.zeros((128, 3 * H2), np.float32)
    w2pk[:, 0:H2] = W2f[0:128]
    w2pk[:, H2:2 * H2] = W2f[128:256]
    w2pk[0:44, 2 * H2:3 * H2] = W2f[256:300]
    w2pk[64:108, 2 * H2:3 * H2] = W2f[256:300]
    w2pk = np.ascontiguousarray(w2pk.astype(bf))
    # w3 zero-padded [100, 112] so L3 stays in full (128,128) PE mode
    w3pk = np.zeros((H2, W3P), np.float32)
    w3pk[:, :H3] = np.asarray(W3, np.float32)
    w3pk = np.ascontiguousarray(w3pk.astype(bf))
    b1f = np.asarray(b1, np.float32)
    bias = np.zeros((128, 5), np.float32)
    bias[0:128, 0] = b1f[0:128]
    bias[0:128, 1] = b1f[128:256]
    bias[0:44, 2] = b1f[256:300]
    bias[64:108, 2] = b1f[256:300]
    bias[:H2, 3] = np.asarray(b2, np.float32)
    bias[:H3, 4] = np.asarray(b3, np.float32)
    in_maps = []
    for c in range(N_CORES):
        in_maps.append({
            "xT": np.ascontiguousarray(xT[:, c * BC:(c + 1) * BC]),
            "fold": foldpk,
            "w2": w2pk, "w3": w3pk,
            "bias": bias,
        })
    return in_maps


def kernel(x, conv_w, W1, b1, W2, b2, W3, b3):
    x = np.asarray(x, dtype=np.float32)
    conv_w = np.asarray(conv_w, dtype=np.float32)

    if "nc" not in _prog_cache:
        _prog_cache["nc"] = _build_program()
    nc = _prog_cache["nc"]

    in_maps = _make_in_maps(x, conv_w, W1, b1, W2, b2, W3, b3)
    res = bass_utils.run_bass_kernel_spmd(
        nc, in_maps, core_ids=list(range(N_CORES))
    )

    out = np.empty((B, H3), np.float32)
    for c in range(N_CORES):
        out[c * BC:(c + 1) * BC, :] = res.results[c]["yT"].T
    return out
